# revision 10
# baseline (speedup 1.0000x reference)
"""HGCN (hyperbolic GCN) 2-layer forward for Trainium2, 8 NeuronCores.

Strategy (graph-parallel, dense-spmm):
  - Nodes padded 10000 -> 10240 and sharded 1280/core (8 cores).
  - segment_sum over the edge list is recast as a dense matmul
    agg = A @ xt with A[dst, src] = sum of edge weights; A is built on the
    host from edge_index/edge_weight and each core streams its
    [10240 x 1280] column slice of A^T (k-major tiles) from HBM.
  - Per layer: HypLinear + logmap0 run on the core's own 1280 nodes,
    the [1280, 256] tangent features are AllGathered (DRAM bounce),
    the spmm accumulates 10 PSUM tiles over 80 k-tiles, and HypAct
    (expmap0/proj/relu-logmap/expmap0/proj) finishes in place.
  - All per-node scalar chains (norms, artanh, tanh, mobius coeffs) are
    batched as [128, 10] column arrays to amortize instruction overhead.

kernel(**inputs) takes the FULL unsharded inputs and returns [2, N, D].
"""

import sys

import numpy as np

for _p in ("/opt/trn_rl_repo",):
    if _p not in sys.path:
        sys.path.append(_p)

import concourse.bass as bass  # noqa: E402
import concourse.tile as tile  # noqa: E402
from concourse import bacc, mybir  # noqa: E402
from concourse.bass_utils import run_bass_kernel_spmd  # noqa: E402
from concourse.masks import make_identity  # noqa: E402

AF = mybir.ActivationFunctionType
ALU = mybir.AluOpType
F32 = mybir.dt.float32

NCORES = 8
N = 10000
D = 256
NP = 10240
PC = NP // NCORES      # 1280 nodes per core
NT = PC // 128         # 10 node tiles per core
KT = NP // 128         # 80 contraction tiles
MAXN = 1.0 - 4e-3      # PROJ_EPS clip for c=1
MINN = 1e-15
MM_DT = "float32"      # dtype of the spmm operands ("float32" | "bfloat16")
DEBUG = False          # add intermediate dumps for layer 0


def _mm_np_dtype():
    if MM_DT == "bfloat16":
        import ml_dtypes

        return np.dtype(ml_dtypes.bfloat16)
    return np.dtype(np.float32)


def _mm_bir_dtype():
    return mybir.dt.bfloat16 if MM_DT == "bfloat16" else F32


def build_nc(y2s):
    """Build the per-core Bass program. y2s = (||hyp_b1||^2, ||hyp_b2||^2)."""
    mmdt = _mm_bir_dtype()
    nc = bacc.Bacc("TRN2", target_bir_lowering=False, debug=False,
                   num_devices=NCORES)

    xc = nc.dram_tensor("xc", [NT, 128, D], F32, kind="ExternalInput")
    a_d = nc.dram_tensor("a", [KT, 128, PC], mmdt, kind="ExternalInput")
    w1t = nc.dram_tensor("w1t", [2, 128, D], F32, kind="ExternalInput")
    w2t = nc.dram_tensor("w2t", [2, 128, D], F32, kind="ExternalInput")
    hb1 = nc.dram_tensor("hb1", [128, D], F32, kind="ExternalInput")
    hb2 = nc.dram_tensor("hb2", [128, D], F32, kind="ExternalInput")
    e1_d = nc.dram_tensor("e1", [NT, 128, D], F32, kind="ExternalOutput")
    e2_d = nc.dram_tensor("e2", [NT, 128, D], F32, kind="ExternalOutput")
    dbg = {}
    if DEBUG:
        for nm, shp in [("dbg_h", [NT, 128, D]), ("dbg_mx", [NT, 128, D]),
                        ("dbg_xt", [NT, 128, D]), ("dbg_agg", [NT, 128, D]),
                        ("dbg_xtf", [KT, 128, D])]:
            dbg[nm] = nc.dram_tensor(nm, shp, F32, kind="ExternalOutput")

    with tile.TileContext(nc) as tc:
        with (
            tc.tile_pool(name="const", bufs=1) as const,
            tc.tile_pool(name="persist", bufs=1) as persist,
            tc.tile_pool(name="sqp", bufs=3) as sqp,
            tc.tile_pool(name="htp", bufs=4) as htp,
            tc.tile_pool(name="atp", bufs=4) as atp,
            tc.tile_pool(name="pst", bufs=2, space="PSUM") as pst,
            tc.tile_pool(name="psmx", bufs=1, space="PSUM") as psmx,
            tc.tile_pool(name="psagg", bufs=1, space="PSUM") as psagg,
            tc.tile_pool(name="dram", bufs=1, space="DRAM") as dram,
        ):
            ident = const.tile([128, 128], F32, name="ident")
            make_identity(nc, ident)

            w_sb = []
            for li, wd in enumerate((w1t, w2t)):
                w = const.tile([128, 2, D], F32, name=f"w{li}")
                nc.sync.dma_start(w[:], wd.ap().rearrange("k p n -> p k n"))
                w_sb.append(w)
            hb_sb = []
            for li, hd in enumerate((hb1, hb2)):
                h = const.tile([128, D], F32, name=f"hb{li}")
                nc.sync.dma_start(h[:], hd.ap())
                hb_sb.append(h)

            def sc(name):
                return persist.tile([128, NT], F32, name=name)

            def square_accum(src_ap, accum_ap, name):
                s = sqp.tile([128, D], F32, name="sqt", tag="sqt")
                nc.scalar.activation(s[:], src_ap, AF.Square, accum_out=accum_ap)

            def clamp_recip(dst, src, name):
                c = sc(name + "_c")
                nc.vector.tensor_scalar_max(c[:], src[:], MINN)
                nc.vector.reciprocal(dst[:], c[:])

            def artanh_ln(dst, x, name):
                """dst = ln((1+x)/(1-x)); caller owns the 0.5 factor."""
                ap1 = sc(name + "_ap")
                am1 = sc(name + "_am")
                ram = sc(name + "_ram")
                q = sc(name + "_q")
                nc.scalar.activation(ap1[:], x[:], AF.Identity, bias=1.0)
                nc.scalar.activation(am1[:], x[:], AF.Identity, bias=1.0, scale=-1.0)
                nc.vector.reciprocal(ram[:], am1[:])
                nc.vector.tensor_tensor(q[:], ap1[:], ram[:], ALU.mult)
                nc.scalar.activation(dst[:], q[:], AF.Ln)

            # ---------------- encode: h = proj(expmap0(x)) ----------------
            x_sb = persist.tile([128, NT, D], F32, name="x_sb", tag="bigA")
            nc.sync.dma_start(x_sb[:], xc.ap().rearrange("t p d -> p t d"))
            h_all = persist.tile([128, NT, D], F32, name="h_all", tag="bigB")
            xn2 = sc("xn2")
            for t in range(NT):
                square_accum(x_sb[:, t, :], xn2[:, t : t + 1], f"enc{t}")
            un = sc("un")
            nc.scalar.activation(un[:], xn2[:], AF.Sqrt)
            run_ = sc("run")
            clamp_recip(run_, un, "enc_r")
            thx = sc("thx")
            nc.scalar.activation(thx[:], un[:], AF.Tanh)
            mn0 = sc("mn0")
            nc.vector.tensor_scalar_min(mn0[:], thx[:], MAXN)
            s0 = sc("s0")
            nc.vector.tensor_tensor(s0[:], mn0[:], run_[:], ALU.mult)
            for t in range(NT):
                nc.vector.tensor_scalar_mul(h_all[:, t, :], x_sb[:, t, :],
                                            s0[:, t : t + 1])

            def layer(li, h_in, hnorm, e_out_d):
                """One HGCN layer; h_in [128,NT,D] on-ball, hnorm [128,NT] its
                row norms. Returns (e_all, out_norms)."""
                L = f"l{li}_"
                w = w_sb[li]
                hb = hb_sb[li]
                y2 = float(y2s[li])

                # ---- HypLinear matmuls + |mx|^2 ----
                mx_all = persist.tile([128, NT, D], F32, name=L + "mx", tag="bigA")
                mn2 = sc(L + "mn2")
                for t in range(NT):
                    hT = htp.tile([128, 2, 128], F32, name="hT", tag="hT")
                    for kc in range(2):
                        psT = pst.tile([128, 128], F32, name="psT", tag="psT")
                        nc.tensor.transpose(
                            psT[:], h_in[:, t, kc * 128 : (kc + 1) * 128], ident[:])
                        nc.vector.tensor_copy(hT[:, kc, :], psT[:])
                    pmx = psmx.tile([128, D], F32, name="pmx", tag="pmx")
                    nc.tensor.matmul(pmx[:], hT[:, 0, :], w[:, 0, :],
                                     start=True, stop=False)
                    nc.tensor.matmul(pmx[:], hT[:, 1, :], w[:, 1, :],
                                     start=False, stop=True)
                    square_accum(pmx[:], mn2[:, t : t + 1], L + f"mx{t}")
                    nc.vector.tensor_copy(mx_all[:, t, :], pmx[:])

                # ---- SB1: mobius_matvec scalars ----
                mxn = sc(L + "mxn")
                nc.scalar.activation(mxn[:], mn2[:], AF.Sqrt)
                nc.vector.tensor_scalar_max(mxn[:], mxn[:], MINN)
                rxn = sc(L + "rxn")
                clamp_recip(rxn, hnorm, L + "rxn")
                rmxn = sc(L + "rmxn")
                nc.vector.reciprocal(rmxn[:], mxn[:])
                atx = sc(L + "atx")
                artanh_ln(atx, hnorm, L + "atx")
                targ = sc(L + "targ")
                nc.vector.tensor_tensor(targ[:], mxn[:], rxn[:], ALU.mult)
                nc.vector.tensor_tensor(targ[:], targ[:], atx[:], ALU.mult)
                th = sc(L + "th")
                nc.scalar.activation(th[:], targ[:], AF.Tanh, scale=0.5)
                sres = sc(L + "sres")
                nc.vector.tensor_tensor(sres[:], th[:], rmxn[:], ALU.mult)
                # proj of res: norm is th (analytically); f1 = min(MAXN/th, 1)
                rth = sc(L + "rth")
                clamp_recip(rth, th, L + "rth")
                f1 = sc(L + "f1")
                nc.vector.tensor_scalar(f1[:], rth[:], MAXN, 1.0, ALU.mult, ALU.min)
                nres = sc(L + "nres")
                nc.vector.tensor_scalar_min(nres[:], th[:], MAXN)
                x2 = sc(L + "x2")
                nc.vector.tensor_tensor(x2[:], nres[:], nres[:], ALU.mult)

                # ---- per tile: xy accumulation (on unscaled mx) ----
                ryp = sc(L + "ryp")
                for t in range(NT):
                    prod = sqp.tile([128, D], F32, name="prodt", tag="prodt")
                    nc.vector.tensor_tensor(prod[:], mx_all[:, t, :], hb[:],
                                            ALU.mult)
                    nc.scalar.activation(prod[:], prod[:], AF.Identity,
                                         accum_out=ryp[:, t : t + 1])

                # ---- SB2: mobius_add coefficients ----
                xy = sc(L + "xy")
                nc.vector.tensor_tensor(xy[:], ryp[:], sres[:], ALU.mult)
                nc.vector.tensor_tensor(xy[:], xy[:], f1[:], ALU.mult)
                apre = sc(L + "apre")
                nc.vector.tensor_scalar(apre[:], xy[:], 2.0, 1.0 + y2,
                                        ALU.mult, ALU.add)
                alpha = sc(L + "alpha")
                nc.vector.tensor_tensor(alpha[:], apre[:], f1[:], ALU.mult)
                beta = sc(L + "beta")
                nc.scalar.activation(beta[:], x2[:], AF.Identity,
                                     bias=1.0, scale=-1.0)
                den = sc(L + "den")
                nc.vector.tensor_scalar(den[:], x2[:], y2, 1.0, ALU.mult, ALU.add)
                xy2 = sc(L + "xy2")
                nc.vector.tensor_scalar_mul(xy2[:], xy[:], 2.0)
                nc.vector.tensor_tensor(den[:], den[:], xy2[:], ALU.add)
                dinv = sc(L + "dinv")
                clamp_recip(dinv, den, L + "dinv")
                asc = sc(L + "asc")
                nc.vector.tensor_tensor(asc[:], alpha[:], dinv[:], ALU.mult)
                nc.vector.tensor_tensor(asc[:], asc[:], sres[:], ALU.mult)
                bsc = sc(L + "bsc")
                nc.vector.tensor_tensor(bsc[:], beta[:], dinv[:], ALU.mult)

                # ---- per tile: h2 = asc*mx + bsc*hb ; |h2|^2 ----
                h2_all = persist.tile([128, NT, D], F32, name=L + "h2", tag="bigB")
                hn2 = sc(L + "hn2")
                for t in range(NT):
                    t1 = sqp.tile([128, D], F32, name="t1t", tag="t1t")
                    nc.vector.tensor_scalar_mul(t1[:], mx_all[:, t, :],
                                                asc[:, t : t + 1])
                    t2 = sqp.tile([128, D], F32, name="t2t", tag="t2t")
                    nc.scalar.activation(t2[:], hb[:], AF.Copy,
                                         scale=bsc[:, t : t + 1])
                    nc.vector.tensor_tensor(h2_all[:, t, :], t1[:], t2[:], ALU.add)
                    square_accum(h2_all[:, t, :], hn2[:, t : t + 1], L + f"h2{t}")

                # ---- SB3: proj + logmap0 scale ----
                hn = sc(L + "hn")
                nc.scalar.activation(hn[:], hn2[:], AF.Sqrt)
                rhn = sc(L + "rhn")
                clamp_recip(rhn, hn, L + "rhn")
                f2 = sc(L + "f2")
                nc.vector.tensor_scalar(f2[:], rhn[:], MAXN, 1.0, ALU.mult, ALU.min)
                m = sc(L + "m")
                nc.vector.tensor_scalar_min(m[:], hn[:], MAXN)
                rm = sc(L + "rm")
                clamp_recip(rm, m, L + "rm")
                atm = sc(L + "atm")
                artanh_ln(atm, m, L + "atm")
                g = sc(L + "g")
                nc.vector.tensor_tensor(g[:], atm[:], rm[:], ALU.mult)
                nc.vector.tensor_tensor(g[:], g[:], f2[:], ALU.mult)
                nc.vector.tensor_scalar_mul(g[:], g[:], 0.5)

                # ---- per tile: xt = g * h2 (tangent features) ----
                mmdt_ = _mm_bir_dtype()
                xt_all = persist.tile([128, NT, D], mmdt_, name=L + "xt", tag="bigC")
                for t in range(NT):
                    nc.vector.tensor_scalar_mul(xt_all[:, t, :], h2_all[:, t, :],
                                                g[:, t : t + 1])

                if DEBUG and li == 0:
                    nc.sync.dma_start(dbg["dbg_mx"].ap().rearrange("t p d -> p t d"),
                                      mx_all[:])
                    nc.sync.dma_start(dbg["dbg_xt"].ap().rearrange("t p d -> p t d"),
                                      xt_all[:])
                # ---- AllGather tangent features ----
                agin = dram.tile([NT, 128, D], mmdt_, name=L + "agin", tag="agin")
                agout = dram.tile([KT, 128, D], mmdt_, name=L + "agout",
                                  tag="agout", addr_space="Shared")
                nc.sync.dma_start(agin[:].rearrange("t p d -> p t d"), xt_all[:])
                nc.gpsimd.collective_compute(
                    "AllGather", ALU.bypass,
                    replica_groups=[list(range(NCORES))],
                    ins=[agin[:].opt()], outs=[agout[:].opt()])
                xt_full = persist.tile([128, KT, D], mmdt_, name="xt_full",
                                       tag="xt_full")
                nc.sync.dma_start(xt_full[:], agout[:].rearrange("t p d -> p t d"))
                if DEBUG and li == 0:
                    nc.sync.dma_start(dbg["dbg_xtf"].ap().rearrange("t p d -> p t d"),
                                      xt_full[:])

                # ---- spmm: agg[dst, f] = sum_src AT[src, dst] xt[src, f] ----
                pagg = psagg.tile([128, NT, D], F32, name="pagg", tag="pagg")
                for kt in range(KT):
                    at_k = atp.tile([128, PC], mmdt_, name="at_k", tag="at_k")
                    nc.sync.dma_start(at_k[:], a_d.ap()[kt])
                    for t in range(NT):
                        # PSUM 'start' clears the whole 2KB bank; tiles t and
                        # t+1 share a bank, so only the even tile's first
                        # matmul may issue start=True.
                        nc.tensor.matmul(
                            pagg[:, t, :],
                            at_k[:, t * 128 : (t + 1) * 128],
                            xt_full[:, kt, :],
                            start=(kt == 0 and t % 2 == 0),
                            stop=(kt == KT - 1),
                            skip_group_check=True)

                # ---- HypAct ----
                if DEBUG and li == 0:
                    agg_sb = persist.tile([128, NT, D], F32, name="agg_sb")
                    for t in range(NT):
                        nc.vector.tensor_copy(agg_sb[:, t, :], pagg[:, t, :])
                    nc.sync.dma_start(dbg["dbg_agg"].ap().rearrange("t p d -> p t d"),
                                      agg_sb[:])
                r2 = sc(L + "r2")
                for t in range(NT):
                    square_accum(pagg[:, t, :], r2[:, t : t + 1], L + f"agg{t}")
                rn = sc(L + "rn")
                nc.scalar.activation(rn[:], r2[:], AF.Sqrt)
                rrn = sc(L + "rrn")
                clamp_recip(rrn, rn, L + "rrn")
                th2 = sc(L + "th2")
                nc.scalar.activation(th2[:], rn[:], AF.Tanh)
                m1 = sc(L + "m1")
                nc.vector.tensor_scalar_min(m1[:], th2[:], MAXN)
                rm1 = sc(L + "rm1")
                clamp_recip(rm1, m1, L + "rm1")
                s1 = sc(L + "s1")
                nc.vector.tensor_tensor(s1[:], m1[:], rrn[:], ALU.mult)
                atq = sc(L + "atq")
                artanh_ln(atq, m1, L + "atq")
                qs = sc(L + "qs")
                nc.vector.tensor_tensor(qs[:], s1[:], atq[:], ALU.mult)
                nc.vector.tensor_tensor(qs[:], qs[:], rm1[:], ALU.mult)
                nc.vector.tensor_scalar_mul(qs[:], qs[:], 0.5)

                xt2_all = persist.tile([128, NT, D], F32, name=L + "xt2", tag="bigD")
                n2b = sc(L + "n2b")
                for t in range(NT):
                    nc.scalar.activation(xt2_all[:, t, :], pagg[:, t, :], AF.Relu,
                                         scale=qs[:, t : t + 1])
                    square_accum(xt2_all[:, t, :], n2b[:, t : t + 1], L + f"xb{t}")

                un2 = sc(L + "un2")
                nc.scalar.activation(un2[:], n2b[:], AF.Sqrt)
                run2 = sc(L + "run2")
                clamp_recip(run2, un2, L + "run2")
                th3 = sc(L + "th3")
                nc.scalar.activation(th3[:], un2[:], AF.Tanh)
                mm2 = sc(L + "mm2")
                nc.vector.tensor_scalar_min(mm2[:], th3[:], MAXN)
                ss = sc(L + "ss")
                nc.vector.tensor_tensor(ss[:], mm2[:], run2[:], ALU.mult)

                e_all = persist.tile([128, NT, D], F32, name=L + "e", tag="bigE")
                for t in range(NT):
                    nc.vector.tensor_scalar_mul(e_all[:, t, :], xt2_all[:, t, :],
                                                ss[:, t : t + 1])
                nc.sync.dma_start(e_out_d.ap().rearrange("t p d -> p t d"),
                                  e_all[:])
                return e_all, mm2

            if DEBUG:
                nc.sync.dma_start(dbg["dbg_h"].ap().rearrange("t p d -> p t d"),
                                  h_all[:])
            e1_all, n1 = layer(0, h_all, mn0, e1_d)
            layer(1, e1_all, n1, e2_d)

    nc.compile()
    return nc


def _hyp_bias(b):
    """proj(expmap0(b, c=1), c=1) in float32, mirroring the reference."""
    b = b.astype(np.float32)
    un = np.maximum(np.sqrt((b * b).sum()), np.float32(MINN)).astype(np.float32)
    h = (np.tanh(un) * b / un).astype(np.float32)
    n = np.maximum(np.sqrt((h * h).sum()), np.float32(MINN)).astype(np.float32)
    if n > np.float32(MAXN):
        h = (h / n * np.float32(MAXN)).astype(np.float32)
    return h


def prepare_inputs(x, W1, b1, W2, b2, edge_index, edge_weight):
    mmnp = _mm_np_dtype()
    x = np.asarray(x, np.float32)
    W1 = np.asarray(W1, np.float32)
    W2 = np.asarray(W2, np.float32)
    b1 = np.asarray(b1, np.float32)
    b2 = np.asarray(b2, np.float32)
    ew = np.asarray(edge_weight, np.float32)
    src = np.asarray(edge_index[0], np.int64)
    dst = np.asarray(edge_index[1], np.int64)

    AT = np.zeros((NP, NP), np.float32)
    np.add.at(AT, (src, dst), ew)

    xfull = np.zeros((NP, D), np.float32)
    xfull[:N] = x

    hb1 = _hyp_bias(b1)
    hb2 = _hyp_bias(b2)
    y2s = (float((hb1.astype(np.float64) ** 2).sum()),
           float((hb2.astype(np.float64) ** 2).sum()))

    w1t = np.ascontiguousarray(W1.T).reshape(2, 128, D)
    w2t = np.ascontiguousarray(W2.T).reshape(2, 128, D)
    hb1_b = np.tile(hb1[None, :], (128, 1)).astype(np.float32)
    hb2_b = np.tile(hb2[None, :], (128, 1)).astype(np.float32)

    in_maps = []
    for c in range(NCORES):
        ac = np.ascontiguousarray(
            AT[:, c * PC : (c + 1) * PC]).reshape(KT, 128, PC).astype(mmnp)
        xcr = xfull[c * PC : (c + 1) * PC].reshape(NT, 128, D)
        in_maps.append({
            "xc": np.ascontiguousarray(xcr),
            "a": ac,
            "w1t": w1t, "w2t": w2t,
            "hb1": hb1_b, "hb2": hb2_b,
        })
    return in_maps, y2s


def assemble(results):
    e1 = np.concatenate([r["e1"].reshape(PC, D) for r in results], 0)[:N]
    e2 = np.concatenate([r["e2"].reshape(PC, D) for r in results], 0)[:N]
    return np.stack([e1, e2], 0).astype(np.float32)


def run(inputs, trace=False):
    in_maps, y2s = prepare_inputs(**inputs)
    nc = build_nc(y2s)
    res = run_bass_kernel_spmd(nc, in_maps, core_ids=list(range(NCORES)),
                               trace=trace)
    return assemble(res.results), res


def kernel(**inputs):
    out, _ = run(inputs, trace=False)
    return out


# revision 11
# speedup vs baseline: 2.0527x; 2.0527x over previous
"""HGCN (hyperbolic GCN) 2-layer forward for Trainium2, 8 NeuronCores.

Strategy (graph-parallel, dense-spmm):
  - Nodes padded 10000 -> 10240 and sharded 1280/core (8 cores).
  - segment_sum over the edge list is recast as a dense matmul
    agg = A @ xt with A[dst, src] = sum of edge weights; A is built on the
    host from edge_index/edge_weight and each core streams its
    [10240 x 1280] column slice of A^T (k-major tiles) from HBM.
  - Per layer: HypLinear + logmap0 run on the core's own 1280 nodes,
    the [1280, 256] tangent features are AllGathered (DRAM bounce),
    the spmm accumulates 10 PSUM tiles over 80 k-tiles, and HypAct
    (expmap0/proj/relu-logmap/expmap0/proj) finishes in place.
  - All per-node scalar chains (norms, artanh, tanh, mobius coeffs) are
    batched as [128, 10] column arrays to amortize instruction overhead.

kernel(**inputs) takes the FULL unsharded inputs and returns [2, N, D].
"""

import sys

import numpy as np

for _p in ("/opt/trn_rl_repo",):
    if _p not in sys.path:
        sys.path.append(_p)

import concourse.bass as bass  # noqa: E402
import concourse.tile as tile  # noqa: E402
from concourse import bacc, mybir  # noqa: E402
from concourse.bass_utils import run_bass_kernel_spmd  # noqa: E402
from concourse.masks import make_identity  # noqa: E402

AF = mybir.ActivationFunctionType
ALU = mybir.AluOpType
F32 = mybir.dt.float32

NCORES = 8
N = 10000
D = 256
NP = 10240
PC = NP // NCORES      # 1280 nodes per core
NT = PC // 128         # 10 node tiles per core
KT = NP // 128         # 80 contraction tiles
MAXN = 1.0 - 4e-3      # PROJ_EPS clip for c=1
MINN = 1e-15
MM_DT = "bfloat16"     # dtype of the spmm operands ("float32" | "bfloat16")
DEBUG = False          # add intermediate dumps for layer 0


def _mm_np_dtype():
    if MM_DT == "bfloat16":
        import ml_dtypes

        return np.dtype(ml_dtypes.bfloat16)
    return np.dtype(np.float32)


def _mm_bir_dtype():
    return mybir.dt.bfloat16 if MM_DT == "bfloat16" else F32


def build_nc(y2s):
    """Build the per-core Bass program. y2s = (||hyp_b1||^2, ||hyp_b2||^2)."""
    mmdt = _mm_bir_dtype()
    nc = bacc.Bacc("TRN2", target_bir_lowering=False, debug=False,
                   num_devices=NCORES)

    xc = nc.dram_tensor("xc", [NT, 128, D], F32, kind="ExternalInput")
    a_d = nc.dram_tensor("a", [KT, 128, PC], mmdt, kind="ExternalInput")
    w1t = nc.dram_tensor("w1t", [2, 128, D], F32, kind="ExternalInput")
    w2t = nc.dram_tensor("w2t", [2, 128, D], F32, kind="ExternalInput")
    hb1 = nc.dram_tensor("hb1", [128, D], F32, kind="ExternalInput")
    hb2 = nc.dram_tensor("hb2", [128, D], F32, kind="ExternalInput")
    e1_d = nc.dram_tensor("e1", [NT, 128, D], F32, kind="ExternalOutput")
    e2_d = nc.dram_tensor("e2", [NT, 128, D], F32, kind="ExternalOutput")
    dbg = {}
    if DEBUG:
        for nm, shp in [("dbg_h", [NT, 128, D]), ("dbg_mx", [NT, 128, D]),
                        ("dbg_xt", [NT, 128, D]), ("dbg_agg", [NT, 128, D]),
                        ("dbg_xtf", [KT, 128, D])]:
            dbg[nm] = nc.dram_tensor(nm, shp, F32, kind="ExternalOutput")

    with tile.TileContext(nc) as tc:
        with (
            tc.tile_pool(name="const", bufs=1) as const,
            tc.tile_pool(name="persist", bufs=1) as persist,
            tc.tile_pool(name="sqp", bufs=3) as sqp,
            tc.tile_pool(name="htp", bufs=4) as htp,
            tc.tile_pool(name="atp", bufs=8) as atp,
            tc.tile_pool(name="pst", bufs=2, space="PSUM") as pst,
            tc.tile_pool(name="psmx", bufs=1, space="PSUM") as psmx,
            tc.tile_pool(name="psagg", bufs=1, space="PSUM") as psagg,
            tc.tile_pool(name="dram", bufs=1, space="DRAM") as dram,
        ):
            ident = const.tile([128, 128], F32, name="ident")
            make_identity(nc, ident)

            w_sb = []
            for li, wd in enumerate((w1t, w2t)):
                w = const.tile([128, 2, D], F32, name=f"w{li}")
                nc.sync.dma_start(w[:], wd.ap().rearrange("k p n -> p k n"))
                w_sb.append(w)
            hb_sb = []
            for li, hd in enumerate((hb1, hb2)):
                h = const.tile([128, D], F32, name=f"hb{li}")
                nc.sync.dma_start(h[:], hd.ap())
                hb_sb.append(h)

            def sc(name):
                return persist.tile([128, NT], F32, name=name)

            def square_accum(src_ap, accum_ap, name):
                s = sqp.tile([128, D], F32, name="sqt", tag="sqt")
                nc.scalar.activation(s[:], src_ap, AF.Square, accum_out=accum_ap)

            def clamp_recip(dst, src, name):
                c = sc(name + "_c")
                nc.vector.tensor_scalar_max(c[:], src[:], MINN)
                nc.vector.reciprocal(dst[:], c[:])

            def artanh_ln(dst, x, name):
                """dst = ln((1+x)/(1-x)); caller owns the 0.5 factor."""
                ap1 = sc(name + "_ap")
                am1 = sc(name + "_am")
                ram = sc(name + "_ram")
                q = sc(name + "_q")
                nc.scalar.activation(ap1[:], x[:], AF.Identity, bias=1.0)
                nc.scalar.activation(am1[:], x[:], AF.Identity, bias=1.0, scale=-1.0)
                nc.vector.reciprocal(ram[:], am1[:])
                nc.vector.tensor_tensor(q[:], ap1[:], ram[:], ALU.mult)
                nc.scalar.activation(dst[:], q[:], AF.Ln)

            # ---------------- encode: h = proj(expmap0(x)) ----------------
            x_sb = persist.tile([128, NT, D], F32, name="x_sb", tag="bigA")
            nc.sync.dma_start(x_sb[:], xc.ap().rearrange("t p d -> p t d"))
            h_all = persist.tile([128, NT, D], F32, name="h_all", tag="bigB")
            xn2 = sc("xn2")
            for t in range(NT):
                square_accum(x_sb[:, t, :], xn2[:, t : t + 1], f"enc{t}")
            un = sc("un")
            nc.scalar.activation(un[:], xn2[:], AF.Sqrt)
            run_ = sc("run")
            clamp_recip(run_, un, "enc_r")
            thx = sc("thx")
            nc.scalar.activation(thx[:], un[:], AF.Tanh)
            mn0 = sc("mn0")
            nc.vector.tensor_scalar_min(mn0[:], thx[:], MAXN)
            s0 = sc("s0")
            nc.vector.tensor_tensor(s0[:], mn0[:], run_[:], ALU.mult)
            for t in range(NT):
                nc.vector.tensor_scalar_mul(h_all[:, t, :], x_sb[:, t, :],
                                            s0[:, t : t + 1])

            def layer(li, h_in, hnorm, e_out_d):
                """One HGCN layer; h_in [128,NT,D] on-ball, hnorm [128,NT] its
                row norms. Returns (e_all, out_norms)."""
                L = f"l{li}_"
                w = w_sb[li]
                hb = hb_sb[li]
                y2 = float(y2s[li])

                # ---- HypLinear matmuls + |mx|^2 ----
                mx_all = persist.tile([128, NT, D], F32, name=L + "mx", tag="bigA")
                mn2 = sc(L + "mn2")
                for t in range(NT):
                    hT = htp.tile([128, 2, 128], F32, name="hT", tag="hT")
                    for kc in range(2):
                        psT = pst.tile([128, 128], F32, name="psT", tag="psT")
                        nc.tensor.transpose(
                            psT[:], h_in[:, t, kc * 128 : (kc + 1) * 128], ident[:])
                        nc.vector.tensor_copy(hT[:, kc, :], psT[:])
                    pmx = psmx.tile([128, D], F32, name="pmx", tag="pmx")
                    nc.tensor.matmul(pmx[:], hT[:, 0, :], w[:, 0, :],
                                     start=True, stop=False)
                    nc.tensor.matmul(pmx[:], hT[:, 1, :], w[:, 1, :],
                                     start=False, stop=True)
                    square_accum(pmx[:], mn2[:, t : t + 1], L + f"mx{t}")
                    nc.vector.tensor_copy(mx_all[:, t, :], pmx[:])

                # ---- SB1: mobius_matvec scalars ----
                mxn = sc(L + "mxn")
                nc.scalar.activation(mxn[:], mn2[:], AF.Sqrt)
                nc.vector.tensor_scalar_max(mxn[:], mxn[:], MINN)
                rxn = sc(L + "rxn")
                clamp_recip(rxn, hnorm, L + "rxn")
                rmxn = sc(L + "rmxn")
                nc.vector.reciprocal(rmxn[:], mxn[:])
                atx = sc(L + "atx")
                artanh_ln(atx, hnorm, L + "atx")
                targ = sc(L + "targ")
                nc.vector.tensor_tensor(targ[:], mxn[:], rxn[:], ALU.mult)
                nc.vector.tensor_tensor(targ[:], targ[:], atx[:], ALU.mult)
                th = sc(L + "th")
                nc.scalar.activation(th[:], targ[:], AF.Tanh, scale=0.5)
                sres = sc(L + "sres")
                nc.vector.tensor_tensor(sres[:], th[:], rmxn[:], ALU.mult)
                # proj of res: norm is th (analytically); f1 = min(MAXN/th, 1)
                rth = sc(L + "rth")
                clamp_recip(rth, th, L + "rth")
                f1 = sc(L + "f1")
                nc.vector.tensor_scalar(f1[:], rth[:], MAXN, 1.0, ALU.mult, ALU.min)
                nres = sc(L + "nres")
                nc.vector.tensor_scalar_min(nres[:], th[:], MAXN)
                x2 = sc(L + "x2")
                nc.vector.tensor_tensor(x2[:], nres[:], nres[:], ALU.mult)

                # ---- per tile: xy accumulation (on unscaled mx) ----
                ryp = sc(L + "ryp")
                for t in range(NT):
                    prod = sqp.tile([128, D], F32, name="prodt", tag="prodt")
                    nc.vector.tensor_tensor(prod[:], mx_all[:, t, :], hb[:],
                                            ALU.mult)
                    nc.scalar.activation(prod[:], prod[:], AF.Identity,
                                         accum_out=ryp[:, t : t + 1])

                # ---- SB2: mobius_add coefficients ----
                xy = sc(L + "xy")
                nc.vector.tensor_tensor(xy[:], ryp[:], sres[:], ALU.mult)
                nc.vector.tensor_tensor(xy[:], xy[:], f1[:], ALU.mult)
                apre = sc(L + "apre")
                nc.vector.tensor_scalar(apre[:], xy[:], 2.0, 1.0 + y2,
                                        ALU.mult, ALU.add)
                alpha = sc(L + "alpha")
                nc.vector.tensor_tensor(alpha[:], apre[:], f1[:], ALU.mult)
                beta = sc(L + "beta")
                nc.scalar.activation(beta[:], x2[:], AF.Identity,
                                     bias=1.0, scale=-1.0)
                den = sc(L + "den")
                nc.vector.tensor_scalar(den[:], x2[:], y2, 1.0, ALU.mult, ALU.add)
                xy2 = sc(L + "xy2")
                nc.vector.tensor_scalar_mul(xy2[:], xy[:], 2.0)
                nc.vector.tensor_tensor(den[:], den[:], xy2[:], ALU.add)
                dinv = sc(L + "dinv")
                clamp_recip(dinv, den, L + "dinv")
                asc = sc(L + "asc")
                nc.vector.tensor_tensor(asc[:], alpha[:], dinv[:], ALU.mult)
                nc.vector.tensor_tensor(asc[:], asc[:], sres[:], ALU.mult)
                bsc = sc(L + "bsc")
                nc.vector.tensor_tensor(bsc[:], beta[:], dinv[:], ALU.mult)

                # ---- per tile: h2 = asc*mx + bsc*hb ; |h2|^2 ----
                h2_all = persist.tile([128, NT, D], F32, name=L + "h2", tag="bigB")
                hn2 = sc(L + "hn2")
                for t in range(NT):
                    t1 = sqp.tile([128, D], F32, name="t1t", tag="t1t")
                    nc.vector.tensor_scalar_mul(t1[:], mx_all[:, t, :],
                                                asc[:, t : t + 1])
                    t2 = sqp.tile([128, D], F32, name="t2t", tag="t2t")
                    nc.scalar.activation(t2[:], hb[:], AF.Copy,
                                         scale=bsc[:, t : t + 1])
                    nc.vector.tensor_tensor(h2_all[:, t, :], t1[:], t2[:], ALU.add)
                    square_accum(h2_all[:, t, :], hn2[:, t : t + 1], L + f"h2{t}")

                # ---- SB3: proj + logmap0 scale ----
                hn = sc(L + "hn")
                nc.scalar.activation(hn[:], hn2[:], AF.Sqrt)
                rhn = sc(L + "rhn")
                clamp_recip(rhn, hn, L + "rhn")
                f2 = sc(L + "f2")
                nc.vector.tensor_scalar(f2[:], rhn[:], MAXN, 1.0, ALU.mult, ALU.min)
                m = sc(L + "m")
                nc.vector.tensor_scalar_min(m[:], hn[:], MAXN)
                rm = sc(L + "rm")
                clamp_recip(rm, m, L + "rm")
                atm = sc(L + "atm")
                artanh_ln(atm, m, L + "atm")
                g = sc(L + "g")
                nc.vector.tensor_tensor(g[:], atm[:], rm[:], ALU.mult)
                nc.vector.tensor_tensor(g[:], g[:], f2[:], ALU.mult)
                nc.vector.tensor_scalar_mul(g[:], g[:], 0.5)

                # ---- per tile: xt = g * h2 (tangent features) ----
                mmdt_ = _mm_bir_dtype()
                xt_all = persist.tile([128, NT, D], mmdt_, name=L + "xt", tag="bigC")
                for t in range(NT):
                    nc.vector.tensor_scalar_mul(xt_all[:, t, :], h2_all[:, t, :],
                                                g[:, t : t + 1])

                if DEBUG and li == 0:
                    nc.sync.dma_start(dbg["dbg_mx"].ap().rearrange("t p d -> p t d"),
                                      mx_all[:])
                    nc.sync.dma_start(dbg["dbg_xt"].ap().rearrange("t p d -> p t d"),
                                      xt_all[:])
                # ---- AllGather tangent features ----
                agin = dram.tile([NT, 128, D], mmdt_, name=L + "agin", tag="agin")
                agout = dram.tile([KT, 128, D], mmdt_, name=L + "agout",
                                  tag="agout", addr_space="Shared")
                nc.sync.dma_start(agin[:].rearrange("t p d -> p t d"), xt_all[:])
                nc.gpsimd.collective_compute(
                    "AllGather", ALU.bypass,
                    replica_groups=[list(range(NCORES))],
                    ins=[agin[:].opt()], outs=[agout[:].opt()])
                xt_full = persist.tile([128, KT, D], mmdt_, name="xt_full",
                                       tag="xt_full")
                nc.sync.dma_start(xt_full[:], agout[:].rearrange("t p d -> p t d"))
                if DEBUG and li == 0:
                    nc.sync.dma_start(dbg["dbg_xtf"].ap().rearrange("t p d -> p t d"),
                                      xt_full[:])

                # ---- spmm: agg[dst, f] = sum_src AT[src, dst] xt[src, f] ----
                pagg = psagg.tile([128, NT, D], F32, name="pagg", tag="pagg")
                for kt in range(KT):
                    at_k = atp.tile([128, PC], mmdt_, name="at_k", tag="at_k")
                    nc.sync.dma_start(at_k[:], a_d.ap()[kt])
                    for t in range(NT):
                        # PSUM 'start' clears the whole 2KB bank; tiles t and
                        # t+1 share a bank, so only the even tile's first
                        # matmul may issue start=True.
                        nc.tensor.matmul(
                            pagg[:, t, :],
                            at_k[:, t * 128 : (t + 1) * 128],
                            xt_full[:, kt, :],
                            start=(kt == 0 and t % 2 == 0),
                            stop=(kt == KT - 1),
                            skip_group_check=True)

                # ---- HypAct ----
                if DEBUG and li == 0:
                    agg_sb = persist.tile([128, NT, D], F32, name="agg_sb")
                    for t in range(NT):
                        nc.vector.tensor_copy(agg_sb[:, t, :], pagg[:, t, :])
                    nc.sync.dma_start(dbg["dbg_agg"].ap().rearrange("t p d -> p t d"),
                                      agg_sb[:])
                r2 = sc(L + "r2")
                for t in range(NT):
                    square_accum(pagg[:, t, :], r2[:, t : t + 1], L + f"agg{t}")
                rn = sc(L + "rn")
                nc.scalar.activation(rn[:], r2[:], AF.Sqrt)
                rrn = sc(L + "rrn")
                clamp_recip(rrn, rn, L + "rrn")
                th2 = sc(L + "th2")
                nc.scalar.activation(th2[:], rn[:], AF.Tanh)
                m1 = sc(L + "m1")
                nc.vector.tensor_scalar_min(m1[:], th2[:], MAXN)
                rm1 = sc(L + "rm1")
                clamp_recip(rm1, m1, L + "rm1")
                s1 = sc(L + "s1")
                nc.vector.tensor_tensor(s1[:], m1[:], rrn[:], ALU.mult)
                atq = sc(L + "atq")
                artanh_ln(atq, m1, L + "atq")
                qs = sc(L + "qs")
                nc.vector.tensor_tensor(qs[:], s1[:], atq[:], ALU.mult)
                nc.vector.tensor_tensor(qs[:], qs[:], rm1[:], ALU.mult)
                nc.vector.tensor_scalar_mul(qs[:], qs[:], 0.5)

                xt2_all = persist.tile([128, NT, D], F32, name=L + "xt2", tag="bigD")
                n2b = sc(L + "n2b")
                for t in range(NT):
                    nc.scalar.activation(xt2_all[:, t, :], pagg[:, t, :], AF.Relu,
                                         scale=qs[:, t : t + 1])
                    square_accum(xt2_all[:, t, :], n2b[:, t : t + 1], L + f"xb{t}")

                un2 = sc(L + "un2")
                nc.scalar.activation(un2[:], n2b[:], AF.Sqrt)
                run2 = sc(L + "run2")
                clamp_recip(run2, un2, L + "run2")
                th3 = sc(L + "th3")
                nc.scalar.activation(th3[:], un2[:], AF.Tanh)
                mm2 = sc(L + "mm2")
                nc.vector.tensor_scalar_min(mm2[:], th3[:], MAXN)
                ss = sc(L + "ss")
                nc.vector.tensor_tensor(ss[:], mm2[:], run2[:], ALU.mult)

                e_all = persist.tile([128, NT, D], F32, name=L + "e", tag="bigE")
                for t in range(NT):
                    nc.vector.tensor_scalar_mul(e_all[:, t, :], xt2_all[:, t, :],
                                                ss[:, t : t + 1])
                nc.sync.dma_start(e_out_d.ap().rearrange("t p d -> p t d"),
                                  e_all[:])
                return e_all, mm2

            if DEBUG:
                nc.sync.dma_start(dbg["dbg_h"].ap().rearrange("t p d -> p t d"),
                                  h_all[:])
            e1_all, n1 = layer(0, h_all, mn0, e1_d)
            layer(1, e1_all, n1, e2_d)

    nc.compile()
    return nc


def _hyp_bias(b):
    """proj(expmap0(b, c=1), c=1) in float32, mirroring the reference."""
    b = b.astype(np.float32)
    un = np.maximum(np.sqrt((b * b).sum()), np.float32(MINN)).astype(np.float32)
    h = (np.tanh(un) * b / un).astype(np.float32)
    n = np.maximum(np.sqrt((h * h).sum()), np.float32(MINN)).astype(np.float32)
    if n > np.float32(MAXN):
        h = (h / n * np.float32(MAXN)).astype(np.float32)
    return h


def prepare_inputs(x, W1, b1, W2, b2, edge_index, edge_weight):
    mmnp = _mm_np_dtype()
    x = np.asarray(x, np.float32)
    W1 = np.asarray(W1, np.float32)
    W2 = np.asarray(W2, np.float32)
    b1 = np.asarray(b1, np.float32)
    b2 = np.asarray(b2, np.float32)
    ew = np.asarray(edge_weight, np.float32)
    src = np.asarray(edge_index[0], np.int64)
    dst = np.asarray(edge_index[1], np.int64)

    AT = np.zeros((NP, NP), np.float32)
    np.add.at(AT, (src, dst), ew)

    xfull = np.zeros((NP, D), np.float32)
    xfull[:N] = x

    hb1 = _hyp_bias(b1)
    hb2 = _hyp_bias(b2)
    y2s = (float((hb1.astype(np.float64) ** 2).sum()),
           float((hb2.astype(np.float64) ** 2).sum()))

    w1t = np.ascontiguousarray(W1.T).reshape(2, 128, D)
    w2t = np.ascontiguousarray(W2.T).reshape(2, 128, D)
    hb1_b = np.tile(hb1[None, :], (128, 1)).astype(np.float32)
    hb2_b = np.tile(hb2[None, :], (128, 1)).astype(np.float32)

    in_maps = []
    for c in range(NCORES):
        ac = np.ascontiguousarray(
            AT[:, c * PC : (c + 1) * PC]).reshape(KT, 128, PC).astype(mmnp)
        xcr = xfull[c * PC : (c + 1) * PC].reshape(NT, 128, D)
        in_maps.append({
            "xc": np.ascontiguousarray(xcr),
            "a": ac,
            "w1t": w1t, "w2t": w2t,
            "hb1": hb1_b, "hb2": hb2_b,
        })
    return in_maps, y2s


def assemble(results):
    e1 = np.concatenate([r["e1"].reshape(PC, D) for r in results], 0)[:N]
    e2 = np.concatenate([r["e2"].reshape(PC, D) for r in results], 0)[:N]
    return np.stack([e1, e2], 0).astype(np.float32)


def run(inputs, trace=False):
    in_maps, y2s = prepare_inputs(**inputs)
    nc = build_nc(y2s)
    res = run_bass_kernel_spmd(nc, in_maps, core_ids=list(range(NCORES)),
                               trace=trace)
    return assemble(res.results), res


def kernel(**inputs):
    out, _ = run(inputs, trace=False)
    return out
